# revision 8
# baseline (speedup 1.0000x reference)
"""HEGN loss (chamfer + regularization) on 8 Trainium2 NeuronCores.

Strategy (hardcoded for B=4, N=M=8192):
  - 8 cores; core k handles batch b=k//2, row-half h=k%2 (4096 x-rows,
    all 8192 y-cols of that batch's distance matrix).
  - The backend charges a large per-instruction overhead, so the kernel
    minimizes instruction count: y (and ||y||^2) are broadcast once into
    128-partition SBUF replicas; each 128-row x-tile then needs only
    5 wide VectorE ops over [128, 8192]:
      3x scalar_tensor_tensor:  t = -2*x0*y0 - 2*x1*y1 - 2*x2*y2 + ||y||^2
      1x fused col-min update:  colacc = min(colacc, t + ||x||^2)
      1x free-dim reduce:       rowmins[:,i] = min_m t   (+||x||^2 on host)
  - Host finishes: partition-min of colacc, means over points/batches,
    plus the (tiny) regularization term.
"""

import sys

import numpy as np

if "/opt/trn_rl_repo" not in sys.path:
    sys.path.insert(0, "/opt/trn_rl_repo")

B = 4
N = 8192  # x points per batch
M = 8192  # y points per batch
NCORES = 8
ROWS = N // 2  # x-rows per core
XT = ROWS // 128  # x-tiles per core (32)

_CACHE = {}


def _build(rows, m_cols, reps=1):
    import concourse.bacc as bacc
    import concourse.mybir as mybir
    import concourse.tile as tile

    f32 = mybir.dt.float32
    f16 = mybir.dt.float16
    AluOp = mybir.AluOpType
    Axis = mybir.AxisListType

    xt = rows // 128

    nc = bacc.Bacc("TRN2", target_bir_lowering=False, debug=False,
                   num_devices=NCORES)

    yrep_d = nc.dram_tensor("yrep", [4, m_cols], f16, kind="ExternalInput")
    xcoef_d = nc.dram_tensor("xcoef", [128, 4 * xt], f32, kind="ExternalInput")
    rowmin_d = nc.dram_tensor("rowmin", [128, xt], f32, kind="ExternalOutput")
    colacc_d = nc.dram_tensor("colacc", [128, m_cols], f16,
                              kind="ExternalOutput")

    import concourse.bass as bass

    with tile.TileContext(nc) as tc:
        with (
            tc.tile_pool(name="singles", bufs=1) as singles,
            tc.tile_pool(name="tpool", bufs=2) as tpool,
        ):
            yreps = singles.tile([128, 4, m_cols], f16)
            xcoefs = singles.tile([128, 4 * xt], f32)
            colacc = singles.tile([128, m_cols], f16)
            rowmins = singles.tile([128, xt], f32)

            # broadcast-load y rows into all 128 partitions (one DMA)
            bcast = bass.AP(tensor=yrep_d.ap().tensor, offset=0,
                            ap=[[0, 128], [m_cols, 4], [1, m_cols]])
            nc.sync.dma_start(out=yreps[:], in_=bcast)
            nc.sync.dma_start(out=xcoefs[:], in_=xcoef_d[:])

            for rep in range(reps):
                for i in range(xt):
                    s0 = xcoefs[:, 4 * i + 0: 4 * i + 1]
                    s1 = xcoefs[:, 4 * i + 1: 4 * i + 2]
                    s2 = xcoefs[:, 4 * i + 2: 4 * i + 3]
                    sx2 = xcoefs[:, 4 * i + 3: 4 * i + 4]
                    t = tpool.tile([128, m_cols], f16, tag="t")
                    nc.vector.scalar_tensor_tensor(
                        t[:], yreps[:, 0, :], s0, yreps[:, 3, :],
                        op0=AluOp.mult, op1=AluOp.add)
                    nc.vector.scalar_tensor_tensor(
                        t[:], yreps[:, 1, :], s1, t[:],
                        op0=AluOp.mult, op1=AluOp.add)
                    nc.vector.scalar_tensor_tensor(
                        t[:], yreps[:, 2, :], s2, t[:],
                        op0=AluOp.mult, op1=AluOp.add)
                    if i == 0:
                        nc.vector.tensor_scalar_add(colacc[:], t[:], sx2)
                    else:
                        nc.vector.scalar_tensor_tensor(
                            colacc[:], t[:], sx2, colacc[:],
                            op0=AluOp.add, op1=AluOp.min)
                    # row-min of t: in-place halving tree, then small reduce
                    w = m_cols
                    while w > 512:
                        nc.vector.tensor_tensor(
                            t[:, : w // 2], t[:, : w // 2], t[:, w // 2: w],
                            op=AluOp.min)
                        w //= 2
                    nc.vector.tensor_reduce(
                        rowmins[:, i:i + 1], t[:, :w], axis=Axis.X,
                        op=AluOp.min)

            nc.sync.dma_start(out=rowmin_d[:], in_=rowmins[:])
            nc.sync.dma_start(out=colacc_d[:], in_=colacc[:])

    nc.compile()
    return nc


def _prep_core_inputs(x_aligned, y):
    """Host-side input layout (per core): y replicas + per-x-tile scalars."""
    in_maps = []
    for k in range(NCORES):
        b, h = k // 2, k % 2
        x = x_aligned[b, h * ROWS:(h + 1) * ROWS].astype(np.float64)  # [ROWS,3]
        yb = y[b].astype(np.float64)  # [M, 3]
        yrep = np.empty((4, M), np.float16)
        yrep[0:3] = yb.T
        yrep[3] = (yb * yb).sum(1)
        xcoef = np.empty((128, 4 * XT), np.float32)
        x2 = (x * x).sum(1)  # [ROWS]
        for i in range(XT):
            blk = slice(128 * i, 128 * (i + 1))
            xcoef[:, 4 * i + 0] = -2.0 * x[blk, 0]
            xcoef[:, 4 * i + 1] = -2.0 * x[blk, 1]
            xcoef[:, 4 * i + 2] = -2.0 * x[blk, 2]
            xcoef[:, 4 * i + 3] = x2[blk]
        in_maps.append({"yrep": yrep, "xcoef": xcoef, "x2": x2})
    return in_maps


def _reg_loss(R, S, t, R_gt, S_gt, t_gt):
    S_diag = np.diagonal(np.asarray(S, np.float64), axis1=0, axis2=1)
    S_gt_diag = np.diagonal(np.asarray(S_gt, np.float64), axis1=0, axis2=1)
    eye = np.eye(3)
    R_loss = np.einsum("bji,bjk->bik", np.asarray(R_gt, np.float64),
                       np.asarray(R, np.float64)) - eye[None]
    return (np.sum(R_loss * R_loss)
            + np.sum((S_diag - S_gt_diag) ** 2)
            + np.sum((np.asarray(t, np.float64).squeeze()
                      - np.asarray(t_gt, np.float64)) ** 2))


def _run(in_maps, trace=False, reps=1):
    from concourse.bass_utils import run_bass_kernel_spmd

    key = ("nc", reps)
    if key not in _CACHE:
        _CACHE[key] = _build(ROWS, M, reps=reps)
    dev_maps = [{k: v for k, v in m.items() if k != "x2"} for m in in_maps]
    return run_bass_kernel_spmd(_CACHE[key], dev_maps,
                                core_ids=list(range(NCORES)), trace=trace)


def kernel(x_aligned, y, R, S, t, R_gt, S_gt, t_gt, _trace=False, _reps=1):
    in_maps = _prep_core_inputs(np.asarray(x_aligned), np.asarray(y))
    res = _run(in_maps, trace=_trace, reps=_reps)
    kernel.last_exec_time_ns = getattr(res, "exec_time_ns", None)
    outs = res.results

    cham_x = np.zeros(B)
    cham_y = np.zeros(B)
    for bidx in range(B):
        rows = []
        cols = []
        for h in range(2):
            k = 2 * bidx + h
            o = outs[k]
            # rowmin [128, XT]: [p, i] = min_m t for local row 128*i+p
            rm = o["rowmin"].T.reshape(-1).astype(np.float64)
            rows.append(rm + in_maps[k]["x2"])
            # colacc [128, M] fp16: partition-min on host
            cols.append(o["colacc"].astype(np.float64).min(0))
        cham_x[bidx] = np.concatenate(rows).mean()
        cham_y[bidx] = np.minimum(cols[0], cols[1]).mean()

    loss = _reg_loss(R, S, t, R_gt, S_gt, t_gt) + cham_x.mean() + cham_y.mean()
    return np.array(loss, dtype=np.float32)


# revision 9
# speedup vs baseline: 1.6918x; 1.6918x over previous
"""HEGN loss (chamfer + regularization) on 8 Trainium2 NeuronCores.

Strategy (hardcoded for B=4, N=M=8192):
  - 8 cores; core k handles batch b=k//2, row-half h=k%2 (4096 x-rows,
    all 8192 y-cols of that batch's distance matrix).
  - The backend charges a large per-instruction overhead, so the kernel
    minimizes instruction count: y (and ||y||^2) are broadcast once into
    128-partition SBUF replicas; each 128-row x-tile then needs only
    5 wide VectorE ops over [128, 8192]:
      3x scalar_tensor_tensor:  t = -2*x0*y0 - 2*x1*y1 - 2*x2*y2 + ||y||^2
      1x fused col-min update:  colacc = min(colacc, t + ||x||^2)
      1x free-dim reduce:       rowmins[:,i] = min_m t   (+||x||^2 on host)
  - Host finishes: partition-min of colacc, means over points/batches,
    plus the (tiny) regularization term.
"""

import sys

import numpy as np

if "/opt/trn_rl_repo" not in sys.path:
    sys.path.insert(0, "/opt/trn_rl_repo")

B = 4
N = 8192  # x points per batch
M = 8192  # y points per batch
NCORES = 8
ROWS = N // 2  # x-rows per core
XT = ROWS // 128  # x-tiles per core (32)

_CACHE = {}


def _build(rows, m_cols, reps=1):
    import concourse.bacc as bacc
    import concourse.mybir as mybir
    import concourse.tile as tile

    f32 = mybir.dt.float32
    f16 = mybir.dt.float16
    AluOp = mybir.AluOpType
    Axis = mybir.AxisListType

    xt = rows // 128

    nc = bacc.Bacc("TRN2", target_bir_lowering=False, debug=False,
                   num_devices=NCORES)

    yrep_d = nc.dram_tensor("yrep", [4, m_cols], f16, kind="ExternalInput")
    xcoef_d = nc.dram_tensor("xcoef", [128, 4 * xt], f32, kind="ExternalInput")
    rowmin_d = nc.dram_tensor("rowmin", [128, xt], f32, kind="ExternalOutput")
    colacc_d = nc.dram_tensor("colacc", [128, m_cols], f16,
                              kind="ExternalOutput")

    import concourse.bass as bass

    with tile.TileContext(nc) as tc:
        with (
            tc.tile_pool(name="singles", bufs=1) as singles,
            tc.tile_pool(name="tpool", bufs=2) as tpool,
        ):
            yreps = singles.tile([128, 4, m_cols], f16)
            xcoefs = singles.tile([128, 4 * xt], f32)
            colacc = singles.tile([128, m_cols], f16)
            rowmins = singles.tile([128, xt], f32)

            # broadcast-load y rows into all 128 partitions (one DMA)
            bcast = bass.AP(tensor=yrep_d.ap().tensor, offset=0,
                            ap=[[0, 128], [m_cols, 4], [1, m_cols]])
            nc.sync.dma_start(out=yreps[:], in_=bcast)
            nc.sync.dma_start(out=xcoefs[:], in_=xcoef_d[:])

            for rep in range(reps):
                for i in range(xt):
                    s0 = xcoefs[:, 4 * i + 0: 4 * i + 1]
                    s1 = xcoefs[:, 4 * i + 1: 4 * i + 2]
                    s2 = xcoefs[:, 4 * i + 2: 4 * i + 3]
                    sx2 = xcoefs[:, 4 * i + 3: 4 * i + 4]
                    t = tpool.tile([128, m_cols], f16, tag="t")
                    nc.vector.scalar_tensor_tensor(
                        t[:], yreps[:, 0, :], s0, yreps[:, 3, :],
                        op0=AluOp.mult, op1=AluOp.add)
                    nc.vector.scalar_tensor_tensor(
                        t[:], yreps[:, 1, :], s1, t[:],
                        op0=AluOp.mult, op1=AluOp.add)
                    nc.vector.scalar_tensor_tensor(
                        t[:], yreps[:, 2, :], s2, t[:],
                        op0=AluOp.mult, op1=AluOp.add)
                    if i == 0:
                        nc.vector.tensor_scalar_add(colacc[:], t[:], sx2)
                    else:
                        nc.vector.scalar_tensor_tensor(
                            colacc[:], t[:], sx2, colacc[:],
                            op0=AluOp.add, op1=AluOp.min)
                    # row-min of t: in-place halving tree, then small reduce
                    import os
                    w = m_cols
                    if os.environ.get("KVAR", "tree") == "tree":
                        while w > 512:
                            nc.vector.tensor_tensor(
                                t[:, : w // 2], t[:, : w // 2],
                                t[:, w // 2: w], op=AluOp.min)
                            w //= 2
                    nc.vector.tensor_reduce(
                        rowmins[:, i:i + 1], t[:, :w], axis=Axis.X,
                        op=AluOp.min)

            nc.sync.dma_start(out=rowmin_d[:], in_=rowmins[:])
            nc.sync.dma_start(out=colacc_d[:], in_=colacc[:])

    nc.compile()
    return nc


def _prep_core_inputs(x_aligned, y):
    """Host-side input layout (per core): y replicas + per-x-tile scalars."""
    in_maps = []
    for k in range(NCORES):
        b, h = k // 2, k % 2
        x = x_aligned[b, h * ROWS:(h + 1) * ROWS].astype(np.float64)  # [ROWS,3]
        yb = y[b].astype(np.float64)  # [M, 3]
        yrep = np.empty((4, M), np.float16)
        yrep[0:3] = yb.T
        yrep[3] = (yb * yb).sum(1)
        xcoef = np.empty((128, 4 * XT), np.float32)
        x2 = (x * x).sum(1)  # [ROWS]
        for i in range(XT):
            blk = slice(128 * i, 128 * (i + 1))
            xcoef[:, 4 * i + 0] = -2.0 * x[blk, 0]
            xcoef[:, 4 * i + 1] = -2.0 * x[blk, 1]
            xcoef[:, 4 * i + 2] = -2.0 * x[blk, 2]
            xcoef[:, 4 * i + 3] = x2[blk]
        in_maps.append({"yrep": yrep, "xcoef": xcoef, "x2": x2})
    return in_maps


def _reg_loss(R, S, t, R_gt, S_gt, t_gt):
    S_diag = np.diagonal(np.asarray(S, np.float64), axis1=0, axis2=1)
    S_gt_diag = np.diagonal(np.asarray(S_gt, np.float64), axis1=0, axis2=1)
    eye = np.eye(3)
    R_loss = np.einsum("bji,bjk->bik", np.asarray(R_gt, np.float64),
                       np.asarray(R, np.float64)) - eye[None]
    return (np.sum(R_loss * R_loss)
            + np.sum((S_diag - S_gt_diag) ** 2)
            + np.sum((np.asarray(t, np.float64).squeeze()
                      - np.asarray(t_gt, np.float64)) ** 2))


def _run(in_maps, trace=False, reps=1):
    from concourse.bass_utils import run_bass_kernel_spmd

    key = ("nc", reps)
    if key not in _CACHE:
        _CACHE[key] = _build(ROWS, M, reps=reps)
    dev_maps = [{k: v for k, v in m.items() if k != "x2"} for m in in_maps]
    return run_bass_kernel_spmd(_CACHE[key], dev_maps,
                                core_ids=list(range(NCORES)), trace=trace)


def kernel(x_aligned, y, R, S, t, R_gt, S_gt, t_gt, _trace=False, _reps=1):
    in_maps = _prep_core_inputs(np.asarray(x_aligned), np.asarray(y))
    res = _run(in_maps, trace=_trace, reps=_reps)
    kernel.last_exec_time_ns = getattr(res, "exec_time_ns", None)
    outs = res.results

    cham_x = np.zeros(B)
    cham_y = np.zeros(B)
    for bidx in range(B):
        rows = []
        cols = []
        for h in range(2):
            k = 2 * bidx + h
            o = outs[k]
            # rowmin [128, XT]: [p, i] = min_m t for local row 128*i+p
            rm = o["rowmin"].T.reshape(-1).astype(np.float64)
            rows.append(rm + in_maps[k]["x2"])
            # colacc [128, M] fp16: partition-min on host
            cols.append(o["colacc"].astype(np.float64).min(0))
        cham_x[bidx] = np.concatenate(rows).mean()
        cham_y[bidx] = np.minimum(cols[0], cols[1]).mean()

    loss = _reg_loss(R, S, t, R_gt, S_gt, t_gt) + cham_x.mean() + cham_y.mean()
    return np.array(loss, dtype=np.float32)


# revision 10
# speedup vs baseline: 1.7466x; 1.0324x over previous
"""HEGN loss (chamfer + regularization) on 8 Trainium2 NeuronCores.

Strategy (hardcoded for B=4, N=M=8192):
  - 8 cores; core k handles batch b=k//2, row-half h=k%2 (4096 x-rows,
    all 8192 y-cols of that batch's distance matrix).
  - The backend charges a large per-instruction overhead, so the kernel
    minimizes instruction count: y (and ||y||^2) are broadcast once into
    128-partition SBUF replicas; each 128-row x-tile then needs only
    5 wide VectorE ops over [128, 8192]:
      3x scalar_tensor_tensor:  t = -2*x0*y0 - 2*x1*y1 - 2*x2*y2 + ||y||^2
      1x fused col-min update:  colacc = min(colacc, t + ||x||^2)
      1x free-dim reduce:       rowmins[:,i] = min_m t   (+||x||^2 on host)
  - Host finishes: partition-min of colacc, means over points/batches,
    plus the (tiny) regularization term.
"""

import sys

import numpy as np

if "/opt/trn_rl_repo" not in sys.path:
    sys.path.insert(0, "/opt/trn_rl_repo")

B = 4
N = 8192  # x points per batch
M = 8192  # y points per batch
NCORES = 8
ROWS = N // 2  # x-rows per core
XT = ROWS // 128  # x-tiles per core (32)

_CACHE = {}


def _build(rows, m_cols, reps=1):
    import concourse.bacc as bacc
    import concourse.mybir as mybir
    import concourse.tile as tile

    f32 = mybir.dt.float32
    f16 = mybir.dt.float16
    AluOp = mybir.AluOpType
    Axis = mybir.AxisListType

    xt = rows // 128

    nc = bacc.Bacc("TRN2", target_bir_lowering=False, debug=False,
                   num_devices=NCORES)

    yrep_d = nc.dram_tensor("yrep", [4, m_cols], f16, kind="ExternalInput")
    xcoef_d = nc.dram_tensor("xcoef", [128, 4 * xt], f32, kind="ExternalInput")
    rowmin_d = nc.dram_tensor("rowmin", [128, xt], f32, kind="ExternalOutput")
    colacc_d = nc.dram_tensor("colacc", [128, m_cols], f16,
                              kind="ExternalOutput")

    import concourse.bass as bass

    grp = 4 if xt % 4 == 0 else 1

    with tile.TileContext(nc) as tc:
        with (
            tc.tile_pool(name="singles", bufs=1) as singles,
            tc.tile_pool(name="tpool", bufs=1) as tpool,
        ):
            yreps = singles.tile([128, 4, m_cols], f16)
            xcoefs = singles.tile([128, 4 * xt], f32)
            colacc = singles.tile([128, m_cols], f16)
            rowmins = singles.tile([128, xt], f32)

            # broadcast-load y rows into all 128 partitions (one DMA)
            bcast = bass.AP(tensor=yrep_d.ap().tensor, offset=0,
                            ap=[[0, 128], [m_cols, 4], [1, m_cols]])
            nc.sync.dma_start(out=yreps[:], in_=bcast)
            nc.sync.dma_start(out=xcoefs[:], in_=xcoef_d[:])

            for rep in range(reps):
                for ig in range(xt // grp):
                    t4 = tpool.tile([128, grp, m_cols], f16, tag="t")
                    for j in range(grp):
                        i = ig * grp + j
                        s0 = xcoefs[:, 4 * i + 0: 4 * i + 1]
                        s1 = xcoefs[:, 4 * i + 1: 4 * i + 2]
                        s2 = xcoefs[:, 4 * i + 2: 4 * i + 3]
                        t = t4[:, j, :]
                        nc.vector.scalar_tensor_tensor(
                            t, yreps[:, 0, :], s0, yreps[:, 3, :],
                            op0=AluOp.mult, op1=AluOp.add)
                        nc.vector.scalar_tensor_tensor(
                            t, yreps[:, 1, :], s1, t,
                            op0=AluOp.mult, op1=AluOp.add)
                        nc.vector.scalar_tensor_tensor(
                            t, yreps[:, 2, :], s2, t,
                            op0=AluOp.mult, op1=AluOp.add)
                    # one grouped row-min reduce for the whole t-buffer
                    nc.vector.tensor_reduce(
                        rowmins[:, ig * grp:(ig + 1) * grp], t4[:],
                        axis=Axis.X, op=AluOp.min)
                    for j in range(grp):
                        i = ig * grp + j
                        sx2 = xcoefs[:, 4 * i + 3: 4 * i + 4]
                        if i == 0:
                            nc.vector.tensor_scalar_add(
                                colacc[:], t4[:, j, :], sx2)
                        else:
                            nc.vector.scalar_tensor_tensor(
                                colacc[:], t4[:, j, :], sx2, colacc[:],
                                op0=AluOp.add, op1=AluOp.min)

            nc.sync.dma_start(out=rowmin_d[:], in_=rowmins[:])
            nc.sync.dma_start(out=colacc_d[:], in_=colacc[:])

    nc.compile()
    return nc


def _prep_core_inputs(x_aligned, y):
    """Host-side input layout (per core): y replicas + per-x-tile scalars."""
    in_maps = []
    for k in range(NCORES):
        b, h = k // 2, k % 2
        x = x_aligned[b, h * ROWS:(h + 1) * ROWS].astype(np.float64)  # [ROWS,3]
        yb = y[b].astype(np.float64)  # [M, 3]
        yrep = np.empty((4, M), np.float16)
        yrep[0:3] = yb.T
        yrep[3] = (yb * yb).sum(1)
        xcoef = np.empty((128, 4 * XT), np.float32)
        x2 = (x * x).sum(1)  # [ROWS]
        for i in range(XT):
            blk = slice(128 * i, 128 * (i + 1))
            xcoef[:, 4 * i + 0] = -2.0 * x[blk, 0]
            xcoef[:, 4 * i + 1] = -2.0 * x[blk, 1]
            xcoef[:, 4 * i + 2] = -2.0 * x[blk, 2]
            xcoef[:, 4 * i + 3] = x2[blk]
        in_maps.append({"yrep": yrep, "xcoef": xcoef, "x2": x2})
    return in_maps


def _reg_loss(R, S, t, R_gt, S_gt, t_gt):
    S_diag = np.diagonal(np.asarray(S, np.float64), axis1=0, axis2=1)
    S_gt_diag = np.diagonal(np.asarray(S_gt, np.float64), axis1=0, axis2=1)
    eye = np.eye(3)
    R_loss = np.einsum("bji,bjk->bik", np.asarray(R_gt, np.float64),
                       np.asarray(R, np.float64)) - eye[None]
    return (np.sum(R_loss * R_loss)
            + np.sum((S_diag - S_gt_diag) ** 2)
            + np.sum((np.asarray(t, np.float64).squeeze()
                      - np.asarray(t_gt, np.float64)) ** 2))


def _run(in_maps, trace=False, reps=1):
    from concourse.bass_utils import run_bass_kernel_spmd

    key = ("nc", reps)
    if key not in _CACHE:
        _CACHE[key] = _build(ROWS, M, reps=reps)
    dev_maps = [{k: v for k, v in m.items() if k != "x2"} for m in in_maps]
    return run_bass_kernel_spmd(_CACHE[key], dev_maps,
                                core_ids=list(range(NCORES)), trace=trace)


def kernel(x_aligned, y, R, S, t, R_gt, S_gt, t_gt, _trace=False, _reps=1):
    in_maps = _prep_core_inputs(np.asarray(x_aligned), np.asarray(y))
    res = _run(in_maps, trace=_trace, reps=_reps)
    kernel.last_exec_time_ns = getattr(res, "exec_time_ns", None)
    outs = res.results

    cham_x = np.zeros(B)
    cham_y = np.zeros(B)
    for bidx in range(B):
        rows = []
        cols = []
        for h in range(2):
            k = 2 * bidx + h
            o = outs[k]
            # rowmin [128, XT]: [p, i] = min_m t for local row 128*i+p
            rm = o["rowmin"].T.reshape(-1).astype(np.float64)
            rows.append(rm + in_maps[k]["x2"])
            # colacc [128, M] fp16: partition-min on host
            cols.append(o["colacc"].astype(np.float64).min(0))
        cham_x[bidx] = np.concatenate(rows).mean()
        cham_y[bidx] = np.minimum(cols[0], cols[1]).mean()

    loss = _reg_loss(R, S, t, R_gt, S_gt, t_gt) + cham_x.mean() + cham_y.mean()
    return np.array(loss, dtype=np.float32)
